# revision 3
# baseline (speedup 1.0000x reference)
"""Trainium2 Bass kernel for 2-layer bipartite GNN propagation (MDCLBR).

Strategy: shard edges by destination row across 8 cores (each core owns a
contiguous slice of output rows per graph). Per 128-row dest tile, edges are
grouped into source-range buckets (int16 gather indices are signed, so each
bucket spans <=32768 source rows). Features are gathered per edge with
dma_gather, scaled one-hot selection matrices are built on the vector engine
(iota + is_equal*val), and the tensor engine accumulates segment sums in PSUM.
Feature tables for the next layer are exchanged with AllGather.
"""
import sys
sys.path.insert(0, '/opt/trn_rl_repo')
import numpy as np

U, I, B, D = 50000, 40000, 20000, 64
NCORES = 8
BUCKET = 32768

_compiled = None


def _layout(rows, cols, vals, n_dest, n_src):
    """Static per-(tile,bucket) chunk layout, maxed across cores (SPMD)."""
    nc_rows = n_dest // NCORES
    T = -(-nc_rows // 128)
    NB = -(-n_src // BUCKET)
    core = rows // nc_rows
    t = (rows % nc_rows) // 128
    b = cols // BUCKET
    key = (core * T + t) * NB + b
    order = np.argsort(key, kind='stable')
    skey = key[order]
    counts = np.bincount(skey, minlength=NCORES * T * NB).reshape(NCORES, T, NB)
    K = -(-counts.max(axis=0) // 128)          # [T, NB] chunks per block
    # group tiles into super-tiles; one gather per (super, bucket) so chunk
    # order is (super, bucket, tile)
    avg = max(1.0, K.sum() / T)
    SUP = max(1, min(16, int(48 // avg)))
    supers = []
    block_off = np.full((T, NB), -1, np.int64)
    choff = 0
    for s0 in range(0, T, SUP):
        ts = range(s0, min(s0 + SUP, T))
        gathers = []
        tiles = []
        for bb in range(NB):
            ktot = int(K[list(ts), bb].sum())
            if ktot > 0:
                gathers.append((bb, ktot, choff))
                for tt in ts:
                    if K[tt, bb] > 0:
                        block_off[tt, bb] = choff
                        choff += int(K[tt, bb])
        for tt in ts:
            tb = [(bb, int(K[tt, bb]), int(block_off[tt, bb]))
                  for bb in range(NB) if K[tt, bb] > 0]
            tiles.append((tt, tb))
        supers.append({'gathers': gathers, 'tiles': tiles})
    C = choff
    idx16 = np.zeros((NCORES, 128, C * 8), np.int16)
    rows_f = np.zeros((NCORES, 128, C), np.float32)
    vals_f = np.zeros((NCORES, 128, C), np.float32)
    # within-group position of each (sorted) edge
    gstart = np.zeros(NCORES * T * NB, np.int64)
    np.cumsum(counts.reshape(-1)[:-1], out=gstart[1:])
    within = np.arange(len(rows)) - gstart[skey]
    so_core, so_t, so_b = core[order], t[order], b[order]
    so_rows = (rows % nc_rows)[order] - so_t * 128
    so_vals = vals[order]
    so_cols = cols[order] - so_b * BUCKET
    cid = block_off[so_t, so_b] + within // 128
    p = within % 128
    rows_f[so_core, p, cid] = so_rows.astype(np.float32)
    vals_f[so_core, p, cid] = so_vals
    col16 = block_off[so_t, so_b] * 8 + within // 16
    prow = within % 16
    for g in range(8):
        idx16[so_core, g * 16 + prow, col16] = so_cols.astype(np.int16)
    return {'T': T, 'NB': NB, 'C': C, 'supers': supers, 'nc_rows': nc_rows,
            'idx16': idx16, 'rows_f': rows_f, 'vals_f': vals_f, 'n_src': n_src}


def _build_program(L_il, L_bl, L_bi):
    from concourse import mybir, bacc
    import concourse.tile as tile

    f32, i16, i32 = mybir.dt.float32, mybir.dt.int16, mybir.dt.int32
    nc = bacc.Bacc("TRN2", target_bir_lowering=False, debug=False,
                   num_devices=NCORES)

    N_il, N_bl = U + I, U + B
    x_il = nc.dram_tensor("x_il", [N_il, D], f32, kind="ExternalInput")
    x_bl = nc.dram_tensor("x_bl", [N_bl, D], f32, kind="ExternalInput")
    x0_il = nc.dram_tensor("x0_il", [L_il['nc_rows'], D], f32, kind="ExternalInput")
    x0_bl = nc.dram_tensor("x0_bl", [L_bl['nc_rows'], D], f32, kind="ExternalInput")
    ins = {}
    for nm, L in (("il", L_il), ("bl", L_bl), ("bi", L_bi)):
        ins[nm] = (
            nc.dram_tensor(f"{nm}_idx", [128, L['C'] * 8], i16, kind="ExternalInput"),
            nc.dram_tensor(f"{nm}_rows", [128, L['C']], f32, kind="ExternalInput"),
            nc.dram_tensor(f"{nm}_vals", [128, L['C']], f32, kind="ExternalInput"),
        )
    il_acc_out = nc.dram_tensor("il_acc_out", [L_il['nc_rows'], D], f32, kind="ExternalOutput")
    bl_acc_out = nc.dram_tensor("bl_acc_out", [L_bl['nc_rows'], D], f32, kind="ExternalOutput")
    bi_out = nc.dram_tensor("bi_out", [L_bi['nc_rows'], D], f32, kind="ExternalOutput")

    il_f1_slice = nc.dram_tensor("il_f1_slice", [L_il['nc_rows'], D], f32)
    il_f1_full = nc.dram_tensor("il_f1_full", [N_il, D], f32, addr_space="Shared")
    il_acc_ag = nc.dram_tensor("il_acc_ag", [L_il['nc_rows'], D], f32)
    il_acc_full = nc.dram_tensor("il_acc_full", [N_il, D], f32, addr_space="Shared")
    bl_f1_slice = nc.dram_tensor("bl_f1_slice", [L_bl['nc_rows'], D], f32)
    bl_f1_full = nc.dram_tensor("bl_f1_full", [N_bl, D], f32, addr_space="Shared")

    RG = [list(range(NCORES))]

    with tile.TileContext(nc) as tc:
        with (
            tc.tile_pool(name="const", bufs=1) as cpool,
            tc.tile_pool(name="meta", bufs=2) as mpool,
            tc.tile_pool(name="idx", bufs=4) as ipool,
            tc.tile_pool(name="gath", bufs=4) as gpool,
            tc.tile_pool(name="sel", bufs=4) as spool,
            tc.tile_pool(name="psum", bufs=4, space="PSUM") as ppool,
            tc.tile_pool(name="feats", bufs=4) as fpool,
            tc.tile_pool(name="nrm", bufs=4) as npool,
            tc.tile_pool(name="acc", bufs=1) as apool,
        ):
            iota_i = cpool.tile([128, 128], i32)
            iota_f = cpool.tile([128, 128], f32)
            nc.gpsimd.iota(iota_i[:], pattern=[[1, 128]], base=0,
                           channel_multiplier=0)
            nc.vector.tensor_copy(iota_f[:], iota_i[:])

            def spmm(L, tensors, x_src, n_src, layer_i, acc_t, x0_dram,
                     feats_out, acc_store=None):
                idx_d, rows_d, vals_d = tensors
                T, C, nc_rows = L['T'], L['C'], L['nc_rows']
                rows_sb = mpool.tile([128, C], f32, tag="rows")
                vals_sb = mpool.tile([128, C], f32, tag="vals")
                nc.sync.dma_start(rows_sb[:], rows_d[:])
                nc.sync.dma_start(vals_sb[:], vals_d[:])
                for sup in L['supers']:
                  gbufs = {}
                  for bb, ktot, goff in sup['gathers']:
                      idx_t = ipool.tile([128, ktot * 8], i16, tag="idx")
                      nc.sync.dma_start(idx_t[:], idx_d[:, goff * 8:(goff + ktot) * 8])
                      g_t = gpool.tile([128, ktot, D], f32, tag="g")
                      base = bb * BUCKET
                      span = min(BUCKET, n_src - base)
                      nc.gpsimd.dma_gather(
                          out_ap=g_t[:], in_ap=x_src[base:base + span, :],
                          idxs_ap=idx_t[:], num_idxs=ktot * 128,
                          num_idxs_reg=ktot * 128, elem_size=D,
                          single_packet=False)
                      gbufs[bb] = (g_t, goff)
                  for tt, tb in sup['tiles']:
                    nchunks = sum(kk for _, kk, _ in tb)
                    psum_t = ppool.tile([128, D], f32, tag="ps")
                    done = 0
                    for bb, kk, off in tb:
                        g_t, goff = gbufs[bb]
                        for k in range(kk):
                            s_t = spool.tile([128, 128], f32, tag="s")
                            nc.vector.tensor_scalar(
                                out=s_t[:], in0=iota_f[:],
                                scalar1=rows_sb[:, off + k:off + k + 1],
                                scalar2=vals_sb[:, off + k:off + k + 1],
                                op0=mybir.AluOpType.is_equal,
                                op1=mybir.AluOpType.mult)
                            nc.tensor.matmul(psum_t[:], s_t[:],
                                             g_t[:, off - goff + k, :],
                                             start=(done == 0),
                                             stop=(done == nchunks - 1))
                            done += 1
                    nrows = min(128, nc_rows - tt * 128)
                    if layer_i is None:
                        # bi aggregation: raw segment sum, no norm
                        o_t = fpool.tile([128, D], f32, tag="f")
                        nc.vector.tensor_copy(o_t[:], psum_t[:])
                        nc.sync.dma_start(
                            feats_out[tt * 128:tt * 128 + nrows, :], o_t[:nrows, :])
                        continue
                    f_t = fpool.tile([128, D], f32, tag="f")
                    nc.scalar.activation(f_t[:], psum_t[:],
                                         mybir.ActivationFunctionType.Copy,
                                         scale=1.0 / (layer_i + 2))
                    sq = npool.tile([128, D], f32, tag="sq")
                    n2 = npool.tile([128, 1], f32, tag="n2")
                    nc.scalar.activation(sq[:], f_t[:],
                                         mybir.ActivationFunctionType.Square,
                                         accum_out=n2[:])
                    nr = npool.tile([128, 1], f32, tag="nr")
                    nc.scalar.activation(nr[:], n2[:],
                                         mybir.ActivationFunctionType.Sqrt)
                    nc.vector.tensor_scalar_max(nr[:], nr[:], 1e-12)
                    ri = npool.tile([128, 1], f32, tag="ri")
                    nc.vector.reciprocal(ri[:], nr[:])
                    aslot = acc_t[:, tt * D:(tt + 1) * D]
                    if layer_i == 0:
                        x0_t = fpool.tile([128, D], f32, tag="x0")
                        nc.sync.dma_start(x0_t[:nrows, :],
                                          x0_dram[tt * 128:tt * 128 + nrows, :])
                        nc.vector.scalar_tensor_tensor(
                            out=aslot, in0=f_t[:], scalar=ri[:, 0:1], in1=x0_t[:],
                            op0=mybir.AluOpType.mult, op1=mybir.AluOpType.add)
                    else:
                        nc.vector.scalar_tensor_tensor(
                            out=aslot, in0=f_t[:], scalar=ri[:, 0:1], in1=aslot,
                            op0=mybir.AluOpType.mult, op1=mybir.AluOpType.add)
                    if feats_out is not None:
                        nc.sync.dma_start(
                            feats_out[tt * 128:tt * 128 + nrows, :], f_t[:nrows, :])
                    if acc_store is not None:
                        for dst in acc_store:
                            nc.sync.dma_start(
                                dst[tt * 128:tt * 128 + nrows, :], aslot[:nrows, :])

            # ---- item-level propagation ----
            acc_il = apool.tile([128, L_il['T'] * D], f32, tag="acc_il")
            spmm(L_il, ins["il"], x_il, N_il, 0, acc_il, x0_il, il_f1_slice)
            nc.gpsimd.collective_compute(
                "AllGather", mybir.AluOpType.bypass, ins=[il_f1_slice[:]],
                outs=[il_f1_full[:]], replica_groups=RG)
            spmm(L_il, ins["il"], il_f1_full, N_il, 1, acc_il, None, None,
                 acc_store=[il_acc_out, il_acc_ag])
            nc.gpsimd.collective_compute(
                "AllGather", mybir.AluOpType.bypass, ins=[il_acc_ag[:]],
                outs=[il_acc_full[:]], replica_groups=RG)
            # ---- bundle-level propagation ----
            acc_bl = apool.tile([128, L_bl['T'] * D], f32, tag="acc_bl")
            spmm(L_bl, ins["bl"], x_bl, N_bl, 0, acc_bl, x0_bl, bl_f1_slice)
            nc.gpsimd.collective_compute(
                "AllGather", mybir.AluOpType.bypass, ins=[bl_f1_slice[:]],
                outs=[bl_f1_full[:]], replica_groups=RG)
            spmm(L_bl, ins["bl"], bl_f1_full, N_bl, 1, acc_bl, None, None,
                 acc_store=[bl_acc_out])
            # ---- bundle-item aggregation from il acc (items section) ----
            spmm(L_bi, ins["bi"], il_acc_full, N_il, None, None, None, bi_out)

    nc.compile()
    return nc


def _install_ntff_hook():
    import importlib.util
    try:
        from antenv.axon_hooks import get_axon_ntff_profile_hook  # noqa
        return True
    except ImportError:
        pass
    try:
        spec = importlib.util.spec_from_file_location(
            "antenv.axon_hooks", "/opt/trn_rl_repo/antenv/axon_hooks.py")
        mod = importlib.util.module_from_spec(spec)
        spec.loader.exec_module(mod)
        sys.modules["antenv.axon_hooks"] = mod
        return True
    except Exception:
        return False


def kernel(users_feature, items_feature, bundles_feature,
           il_rows, il_cols, il_vals,
           bl_rows, bl_cols, bl_vals,
           bi_rows, bi_cols, bi_vals):
    global _compiled
    import os
    from concourse.bass_utils import run_bass_kernel_spmd

    x_il = np.concatenate([np.asarray(users_feature), np.asarray(items_feature)], 0).astype(np.float32)
    x_bl = np.concatenate([np.asarray(users_feature), np.asarray(bundles_feature)], 0).astype(np.float32)
    L_il = _layout(np.asarray(il_rows).astype(np.int64), np.asarray(il_cols).astype(np.int64),
                   np.asarray(il_vals).astype(np.float32), U + I, U + I)
    L_bl = _layout(np.asarray(bl_rows).astype(np.int64), np.asarray(bl_cols).astype(np.int64),
                   np.asarray(bl_vals).astype(np.float32), U + B, U + B)
    L_bi = _layout(np.asarray(bi_rows).astype(np.int64),
                   np.asarray(bi_cols).astype(np.int64) + U,
                   np.asarray(bi_vals).astype(np.float32), B, U + I)

    nc = _build_program(L_il, L_bl, L_bi)

    in_maps = []
    for c in range(NCORES):
        m = {"x_il": x_il, "x_bl": x_bl,
             "x0_il": x_il[c * L_il['nc_rows']:(c + 1) * L_il['nc_rows']],
             "x0_bl": x_bl[c * L_bl['nc_rows']:(c + 1) * L_bl['nc_rows']]}
        for nm, L in (("il", L_il), ("bl", L_bl), ("bi", L_bi)):
            m[f"{nm}_idx"] = L['idx16'][c]
            m[f"{nm}_rows"] = L['rows_f'][c]
            m[f"{nm}_vals"] = L['vals_f'][c]
        in_maps.append(m)

    tkw = {}
    if os.environ.get("KTRACE") == "1" and _install_ntff_hook():
        tkw = dict(trace=True, tmpdir=os.environ.get("KTRACE_DIR", "/tmp/ktrace"))
    res = run_bass_kernel_spmd(nc, in_maps, core_ids=list(range(NCORES)), **tkw)
    kernel.last_exec_ns = res.exec_time_ns

    il_acc = np.concatenate([res.results[c]["il_acc_out"] for c in range(NCORES)], 0)
    bl_acc = np.concatenate([res.results[c]["bl_acc_out"] for c in range(NCORES)], 0)
    bi_o = np.concatenate([res.results[c]["bi_out"] for c in range(NCORES)], 0)
    return np.concatenate([il_acc[:U], bl_acc[:U], bi_o, bl_acc[U:]], 0)

